# revision 1
# baseline (speedup 1.0000x reference)
"""Trainium2 Bass kernel for nn_MinkUNet (sparse voxel UNet stem + residual block).

Self-contained: builds and runs 5 SPMD bass modules on 8 NeuronCores.
  1. vox    : per-voxel mean of point features (padded gather + matmul)
  2. conv1  : sparse 3x3x3 conv (Cin=4) + BN + ReLU
  3. conv   : sparse 3x3x3 conv (Cin=32) + BN + ReLU   (used for stem conv2 and r1)
  4. convr2 : sparse conv + BN + residual + ReLU, fused classifier table Y = h3 @ Wc
  5. devox  : trilinear 8-neighbor gather of Y + bias

Sharding: voxels/points split evenly across 8 cores; gather tables are
replicated (full) per core; BN statistics all-reduced on device; activation
tables are assembled (concat of shards) on host between launches.
"""
import numpy as np

import concourse.bass as bass
import concourse.mybir as mybir
from concourse.tile import TileContext
from concourse.masks import make_identity

f32 = mybir.dt.float32
i32 = mybir.dt.int32
ACT = mybir.ActivationFunctionType
ALU = mybir.AluOpType

# problem sizes (hardcoded per contract)
N, M, K, KD = 400000, 300000, 27, 8
CIN, C0, NCLS = 4, 32, 19
EPS = 1e-5
NC = 8
Ms = M // NC                      # 37500
MsP = 296 * 128                   # 37888 = 74*512
MT = NC * MsP                     # 303104
Np = N // NC                      # 50000
NpP = 392 * 128                   # 50176 = 98*512
ZR = Ms                           # zero row (shard-0 pad row 0) in padded table coords
SUP = 4                           # tiles per supertile
NSUP_V = MsP // (SUP * 128)       # 74
NSUP_P = NpP // (SUP * 128)       # 98
RG = [list(range(NC))]

_cache = {}
LAUNCH_TIMES = []


# ---------------------------------------------------------------- wait splitting
def _split_sync_waits(bir_bytes, wait_limit=1):
    """Pinned walrus encodes at most 1 sync wait per instruction; split extras
    onto same-engine reg-move nops placed immediately before (same program
    order on the engine, semantically identical)."""
    import json
    m = json.loads(bir_bytes)
    ctr = [0]

    def nop(engine, on_wait):
        ctr[0] += 1
        return {
            "debug": 0, "engine": engine,
            "ins": [{"dtype": "int32", "kind": "imm_value", "value": 0}],
            "outs": [{"dtype": "int32", "kind": "register_access",
                      "regref": f"{engine}_zero"}],
            "name": f"wsplit-{ctr[0]}", "opcode": "RegisterMove",
            "sync_info": {"on_wait": on_wait, "on_update": []},
        }

    for f in m["functions"]:
        for b in f["blocks"]:
            out = []
            for ins in b["instructions"]:
                si = ins.get("sync_info")
                if si:
                    ow = si.get("on_wait") or []
                    if len(ow) > wait_limit:
                        extra, keep = ow[:-wait_limit], ow[-wait_limit:]
                        for i in range(0, len(extra), wait_limit):
                            out.append(nop(ins["engine"], extra[i:i + wait_limit]))
                        si["on_wait"] = keep
                out.append(ins)
            b["instructions"] = out
    return json.dumps(m).encode()


def _install_waitfix(nc):
    orig = nc.to_json_bytes
    nc.to_json_bytes = lambda: _split_sync_waits(orig())
    return nc


# ---------------------------------------------------------------- SPMD runner
class _Runner:
    """jit once; inputs device_put per call; mirrors bass2jax multi-core path."""

    def __init__(self, nc):
        import jax
        from jax.sharding import Mesh, PartitionSpec, NamedSharding
        from jax.experimental.shard_map import shard_map
        from concourse import bass2jax
        from concourse.bass2jax import _bass_exec_p, install_neuronx_cc_hook
        install_neuronx_cc_hook()
        self.jax = jax
        self.nc = nc
        pname = nc.partition_id_tensor.name if nc.partition_id_tensor else None
        in_names, out_names, out_avals, zero_shapes = [], [], [], []
        for alloc in nc.m.functions[0].allocations:
            if not isinstance(alloc, mybir.MemoryLocationSet):
                continue
            name = alloc.memorylocations[0].name
            if alloc.kind == "ExternalInput":
                if name != pname:
                    in_names.append(name)
            elif alloc.kind == "ExternalOutput":
                out_names.append(name)
                shape = tuple(alloc.tensor_shape)
                dtype = mybir.dt.np(alloc.dtype)
                out_avals.append(jax.core.ShapedArray(shape, dtype))
                zero_shapes.append((shape, dtype))
        self.in_names, self.out_names, self.out_avals = in_names, out_names, out_avals
        all_in = list(in_names) + list(out_names)
        if pname is not None:
            all_in.append(pname)
        n_params, n_outs = len(in_names), len(out_names)

        def _body(*args):
            operands = list(args)
            if pname is not None:
                operands.append(bass2jax.partition_id_tensor())
            return tuple(_bass_exec_p.bind(
                *operands, out_avals=tuple(out_avals), in_names=tuple(all_in),
                out_names=tuple(out_names), lowering_input_output_aliases=(),
                sim_require_finite=True, sim_require_nnan=True, nc=nc))

        devices = jax.devices()[:NC]
        self.mesh = Mesh(np.asarray(devices), ("core",))
        specs_in = (PartitionSpec("core"),) * (n_params + n_outs)
        specs_out = (PartitionSpec("core"),) * n_outs
        self.fn = jax.jit(
            shard_map(_body, mesh=self.mesh, in_specs=specs_in,
                      out_specs=specs_out, check_rep=False),
            keep_unused=True)
        self.sharding = NamedSharding(self.mesh, PartitionSpec("core"))
        self.zeros = [
            self.jax.device_put(
                np.zeros((NC * s[0], *s[1:]), d), self.sharding)
            for s, d in zero_shapes
        ]

    def __call__(self, in_maps):
        concat = [
            np.concatenate([np.asarray(in_maps[c][n]) for c in range(NC)], 0)
            for n in self.in_names
        ]
        args = [self.jax.device_put(a, self.sharding) for a in concat]
        self.jax.block_until_ready(args)
        import time as _time
        _t0 = _time.perf_counter()
        outs = self.fn(*args, *self.zeros)
        self.jax.block_until_ready(outs)
        LAUNCH_TIMES.append(_time.perf_counter() - _t0)
        res = []
        for c in range(NC):
            res.append({
                n: np.asarray(outs[i]).reshape(NC, *self.out_avals[i].shape)[c]
                for i, n in enumerate(self.out_names)
            })
        return res


# ---------------------------------------------------------------- module builders
_gq = [0]


def _gather(nc, out_ap, table_ap, idx_col):
    inst = nc.gpsimd.indirect_dma_start(
        out=out_ap, out_offset=None, in_=table_ap,
        in_offset=bass.IndirectOffsetOnAxis(ap=idx_col, axis=0))
    q = _gq[0] % 4
    _gq[0] += 1
    if q:
        inst.ins.queue = f"qPoolDynamic{q}"


def _bn_affine(nc, pool, st, g_sb, b_sb):
    """st [32,2] (sum, sumsq over M) -> (a, bb) [32,1] tiles."""
    mean = pool.tile([32, 1], f32, name="bn_mean")
    ex2 = pool.tile([32, 1], f32, name="bn_ex2")
    nc.vector.tensor_scalar_mul(mean[:], st[:, 0:1], 1.0 / M)
    nc.vector.tensor_scalar_mul(ex2[:], st[:, 1:2], 1.0 / M)
    m2 = pool.tile([32, 1], f32, name="bn_m2")
    nc.vector.tensor_tensor(out=m2[:], in0=mean[:], in1=mean[:], op=ALU.mult)
    var = pool.tile([32, 1], f32, name="bn_var")
    nc.vector.tensor_tensor(out=var[:], in0=ex2[:], in1=m2[:], op=ALU.subtract)
    vp = pool.tile([32, 1], f32, name="bn_vp")
    nc.vector.tensor_scalar_add(vp[:], var[:], EPS)
    std = pool.tile([32, 1], f32, name="bn_std")
    nc.scalar.activation(out=std[:], in_=vp[:], func=ACT.Sqrt)
    inv = pool.tile([32, 1], f32, name="bn_inv")
    nc.vector.reciprocal(inv[:], std[:])
    a = pool.tile([32, 1], f32, name="bn_a")
    nc.vector.tensor_tensor(out=a[:], in0=g_sb[:], in1=inv[:], op=ALU.mult)
    ma = pool.tile([32, 1], f32, name="bn_ma")
    nc.vector.tensor_tensor(out=ma[:], in0=mean[:], in1=a[:], op=ALU.mult)
    bb = pool.tile([32, 1], f32, name="bn_bb")
    nc.vector.tensor_tensor(out=bb[:], in0=b_sb[:], in1=ma[:], op=ALU.subtract)
    return a, bb


def build_vox(dmax):
    nc = bass.Bass(num_swdge_queues=4)
    pf = nc.declare_dram_parameter("pf", [N + 1, CIN], f32, isOutput=False)
    vmap = nc.declare_dram_parameter("vmap", [MsP, dmax], i32, isOutput=False)
    rcp = nc.declare_dram_parameter("rcp", [MsP, 1], f32, isOutput=False)
    smat = nc.declare_dram_parameter("smat", [dmax * CIN, CIN], f32, isOutput=False)
    vout = nc.declare_dram_parameter("vout", [MsP, CIN], f32, isOutput=True)
    GW = dmax * CIN
    with TileContext(nc) as tc:
        with (
            tc.tile_pool(name="const", bufs=1) as cp,
            tc.tile_pool(name="sb", bufs=3) as sb,
            tc.tile_pool(name="pp", bufs=2, space="PSUM") as pp,
        ):
            ident = cp.tile([128, 128], f32)
            make_identity(nc, ident[:])
            ssb = cp.tile([GW, CIN], f32)
            nc.sync.dma_start(out=ssb[:], in_=smat[:])
            vmap_r = vmap[:].rearrange("(s t p) k -> s p t k", t=SUP, p=128)
            rcp_r = rcp[:].rearrange("(s t p) o -> s p t o", t=SUP, p=128)
            vout_r = vout[:].rearrange("(s t p) c -> s p t c", t=SUP, p=128)
            for s in range(NSUP_V):
                idx = sb.tile([128, SUP * dmax], i32, name="idx", tag="idx")
                nc.sync.dma_start(
                    out=idx[:].rearrange("p (t k) -> p t k", t=SUP),
                    in_=vmap_r[s])
                G = sb.tile([128, SUP * GW], f32, name="G", tag="G")
                for t in range(SUP):
                    for k in range(dmax):
                        _gather(nc, G[:, t * GW + k * CIN: t * GW + (k + 1) * CIN],
                                pf[:], idx[:, t * dmax + k: t * dmax + k + 1])
                pgt = pp.tile([128, 512], f32, name="pgt", tag="pgt")
                for t in range(SUP):
                    nc.tensor.transpose(out=pgt[:GW, t * 128:(t + 1) * 128],
                                        in_=G[:, t * GW:(t + 1) * GW],
                                        identity=ident[:])
                GT = sb.tile([128, 512], f32, name="GT", tag="GT")
                nc.vector.tensor_copy(out=GT[:GW, :], in_=pgt[:GW, :])
                pv = pp.tile([128, SUP * CIN], f32, name="pv", tag="pv")
                for t in range(SUP):
                    nc.tensor.matmul(out=pv[:, t * CIN:(t + 1) * CIN],
                                     lhsT=GT[:GW, t * 128:(t + 1) * 128],
                                     rhs=ssb[:], start=True, stop=True)
                rc = sb.tile([128, SUP], f32, name="rc", tag="rc")
                nc.sync.dma_start(out=rc[:].rearrange("p (t o) -> p t o", t=SUP),
                                  in_=rcp_r[s])
                vsb = sb.tile([128, SUP * CIN], f32, name="vsb", tag="vsb")
                rcb = bass.AP(rc[:].tensor, rc[:].offset,
                              [list(rc[:].ap[0]), [1, SUP], [0, CIN]])
                nc.vector.tensor_tensor(
                    out=vsb[:].rearrange("p (t c) -> p t c", t=SUP),
                    in0=pv[:].rearrange("p (t c) -> p t c", t=SUP),
                    in1=rcb, op=ALU.mult)
                nc.sync.dma_start(out=vout_r[s],
                                  in_=vsb[:].rearrange("p (t c) -> p t c", t=SUP))
    return _install_waitfix(nc)


def _conv_common(nc, tc, cp, table, nbrs, wst, gpar, bpar, cin_cols, nchunk):
    """Pass A: raw conv -> rawT dram [32, MsP]; returns (rawT, a, bb) after BN."""
    KK = 27                       # idx cols per tile
    GW = KK * cin_cols            # gather row block per tile
    st_in = nc.dram_tensor("st_in", [32, 2], f32)
    st_out = nc.dram_tensor("st_out", [32, 2], f32, addr_space="Shared")
    rawT = nc.dram_tensor("rawT", [32, MsP], f32)

    sp = cp
    with (
        tc.tile_pool(name="sbA", bufs=3) as sb,
        tc.tile_pool(name="ppA", bufs=2, space="PSUM") as pp,
    ):
        ident = cp.tile([128, 128], f32, name="identA")
        make_identity(nc, ident[:])
        wsb = cp.tile([128, nchunk * C0], f32, name="wsb")
        nc.sync.dma_start(
            out=wsb[:].rearrange("p (j c) -> p j c", j=nchunk),
            in_=wst[:].rearrange("(j p) c -> p j c", p=128))
        sums = sp.tile([32, NSUP_V], f32, name="sums")
        sqs = sp.tile([32, NSUP_V], f32, name="sqs")
        nbrs_r = nbrs[:].rearrange("(s t p) k -> s p t k", t=SUP, p=128)
        for s in range(NSUP_V):
            idx = sb.tile([128, SUP * KK], i32, name="idxA", tag="idxA")
            nc.sync.dma_start(
                out=idx[:].rearrange("p (t k) -> p t k", t=SUP),
                in_=nbrs_r[s])
            G = sb.tile([128, SUP * GW], f32, name="GA", tag="GA")
            for t in range(SUP):
                for k in range(KK):
                    _gather(nc, G[:, t * GW + k * cin_cols: t * GW + (k + 1) * cin_cols],
                            table[:], idx[:, t * KK + k: t * KK + k + 1])
            po = pp.tile([32, 512], f32, name="poA", tag="poA")
            for j in range(nchunk):
                pgt = pp.tile([128, 512], f32, name="pgtA", tag="pgtA")
                cw = min(128, GW - j * 128)
                if cw < 128:
                    nc.vector.memset(pgt[:], 0.0)
                for t in range(SUP):
                    nc.tensor.transpose(
                        out=pgt[:cw, t * 128:(t + 1) * 128],
                        in_=G[:, t * GW + j * 128: t * GW + j * 128 + cw],
                        identity=ident[:])
                GT = sb.tile([128, 512], f32, name="GTA", tag="GTA")
                nc.vector.tensor_copy(out=GT[:], in_=pgt[:])
                nc.tensor.matmul(out=po[:], lhsT=wsb[:, j * C0:(j + 1) * C0],
                                 rhs=GT[:], start=(j == 0), stop=(j == nchunk - 1))
            rawsb = sb.tile([32, 512], f32, name="rawA", tag="rawA")
            nc.scalar.activation(out=rawsb[:], in_=po[:], func=ACT.Copy,
                                 accum_out=sums[:, s:s + 1])
            sqsb = sb.tile([32, 512], f32, name="sqA", tag="sqA")
            nc.vector.tensor_tensor(out=sqsb[:], in0=rawsb[:], in1=rawsb[:],
                                    op=ALU.mult)
            nc.vector.tensor_reduce(out=sqs[:, s:s + 1], in_=sqsb[:],
                                    axis=mybir.AxisListType.X, op=ALU.add)
            nc.sync.dma_start(out=rawT[:, s * 512:(s + 1) * 512], in_=rawsb[:])

        stats = sp.tile([32, 2], f32, name="stats")
        nc.vector.tensor_reduce(out=stats[:, 0:1], in_=sums[:],
                                axis=mybir.AxisListType.X, op=ALU.add)
        nc.vector.tensor_reduce(out=stats[:, 1:2], in_=sqs[:],
                                axis=mybir.AxisListType.X, op=ALU.add)
        nc.sync.dma_start(out=st_in[:], in_=stats[:])
        nc.gpsimd.collective_compute("AllReduce", ALU.add, RG,
                                     ins=[st_in[:]], outs=[st_out[:]])
        star = sp.tile([32, 2], f32, name="star")
        nc.sync.dma_start(out=star[:], in_=st_out[:])
        gsb = sp.tile([32, 1], f32, name="gsb")
        bsb = sp.tile([32, 1], f32, name="bsb")
        nc.sync.dma_start(out=gsb[:], in_=gpar[:, None])
        nc.sync.dma_start(out=bsb[:], in_=bpar[:, None])
        a, bb = _bn_affine(nc, sp, star, gsb, bsb)
    return rawT, a, bb, ident


def build_conv(cin_cols, residual):
    """cin_cols: table row width (4 for conv1, 32 for others).
    residual: r2 variant (adds h2, relu, emits Y = h3 @ Wc instead of h)."""
    nchunk = (27 * cin_cols + 127) // 128      # 1 for cin=4, 7 for cin=32
    nc = bass.Bass(num_swdge_queues=4)
    table = nc.declare_dram_parameter("table", [MT, cin_cols], f32, isOutput=False)
    nbrs = nc.declare_dram_parameter("nbrs", [MsP, 27], i32, isOutput=False)
    wst = nc.declare_dram_parameter("wst", [nchunk * 128, C0], f32, isOutput=False)
    gpar = nc.declare_dram_parameter("gpar", [C0], f32, isOutput=False)
    bpar = nc.declare_dram_parameter("bpar", [C0], f32, isOutput=False)
    if residual:
        h2in = nc.declare_dram_parameter("h2in", [MsP, C0], f32, isOutput=False)
        wc = nc.declare_dram_parameter("wc", [C0, C0], f32, isOutput=False)
    hout = nc.declare_dram_parameter("hout", [MsP, C0], f32, isOutput=True)

    with TileContext(nc) as tc:
        with tc.tile_pool(name="const", bufs=1) as cp:
            rawT, a, bb, ident = _conv_common(
                nc, tc, cp, table, nbrs, wst, gpar, bpar, cin_cols, nchunk)
            # pass B
            with (
                tc.tile_pool(name="sbB", bufs=3) as sb,
                tc.tile_pool(name="ppB", bufs=2, space="PSUM") as pp,
            ):
                if residual:
                    wcsb = cp.tile([C0, C0], f32, name="wcsb")
                    nc.sync.dma_start(out=wcsb[:], in_=wc[:])
                    h2_r = h2in[:].rearrange("(s t p) c -> s p t c", t=SUP, p=128)
                hout_r = hout[:].rearrange("(s t p) c -> s p t c", t=SUP, p=128)
                for s in range(NSUP_V):
                    raw2 = sb.tile([32, 512], f32, name="raw2", tag="raw2")
                    nc.sync.dma_start(out=raw2[:], in_=rawT[:, s * 512:(s + 1) * 512])
                    if not residual:
                        hT = sb.tile([32, 512], f32, name="hT", tag="hT")
                        nc.scalar.activation(out=hT[:], in_=raw2[:], func=ACT.Relu,
                                             bias=bb[:], scale=a[:])
                        ph = pp.tile([128, 128], f32, name="ph", tag="ph")
                        for t in range(SUP):
                            nc.tensor.transpose(out=ph[:, t * C0:(t + 1) * C0],
                                                in_=hT[:, t * 128:(t + 1) * 128],
                                                identity=ident[:32, :32])
                        hsb = sb.tile([128, 128], f32, name="hsb", tag="hsb")
                        nc.vector.tensor_copy(out=hsb[:], in_=ph[:])
                        nc.sync.dma_start(
                            out=hout_r[s],
                            in_=hsb[:].rearrange("p (t c) -> p t c", t=SUP))
                    else:
                        t0 = sb.tile([32, 512], f32, name="t0", tag="t0")
                        nc.scalar.activation(out=t0[:], in_=raw2[:], func=ACT.Identity,
                                             bias=bb[:], scale=a[:])
                        h2sb = sb.tile([128, 128], f32, name="h2sb", tag="h2sb")
                        nc.sync.dma_start(
                            out=h2sb[:].rearrange("p (t c) -> p t c", t=SUP),
                            in_=h2_r[s])
                        ph2 = pp.tile([32, 512], f32, name="ph2", tag="ph2")
                        for t in range(SUP):
                            nc.tensor.transpose(out=ph2[:, t * 128:(t + 1) * 128],
                                                in_=h2sb[:, t * C0:(t + 1) * C0],
                                                identity=ident[:])
                        s1 = sb.tile([32, 512], f32, name="s1", tag="s1")
                        nc.vector.tensor_tensor(out=s1[:], in0=t0[:], in1=ph2[:],
                                                op=ALU.add)
                        h3 = sb.tile([32, 512], f32, name="h3", tag="h3")
                        nc.vector.tensor_scalar_max(h3[:], s1[:], 0.0)
                        py = pp.tile([128, 128], f32, name="py", tag="py")
                        for t in range(SUP):
                            nc.tensor.matmul(out=py[:, t * C0:(t + 1) * C0],
                                             lhsT=h3[:, t * 128:(t + 1) * 128],
                                             rhs=wcsb[:], start=True, stop=True)
                        ysb = sb.tile([128, 128], f32, name="ysb", tag="ysb")
                        nc.vector.tensor_copy(out=ysb[:], in_=py[:])
                        nc.sync.dma_start(
                            out=hout_r[s],
                            in_=ysb[:].rearrange("p (t c) -> p t c", t=SUP))
    return _install_waitfix(nc)


def build_devox():
    nc = bass.Bass(num_swdge_queues=4)
    ytab = nc.declare_dram_parameter("ytab", [MT, C0], f32, isOutput=False)
    didx = nc.declare_dram_parameter("didx", [NpP, KD], i32, isOutput=False)
    wdev = nc.declare_dram_parameter("wdev", [NpP, KD], f32, isOutput=False)
    bc = nc.declare_dram_parameter("bc", [1, C0], f32, isOutput=False)
    out = nc.declare_dram_parameter("out", [NpP, NCLS], f32, isOutput=True)
    with TileContext(nc) as tc:
        with (
            tc.tile_pool(name="const", bufs=1) as cp,
            tc.tile_pool(name="sb", bufs=3) as sb,
            tc.tile_pool(name="pp", bufs=2, space="PSUM") as pp,
        ):
            ones = cp.tile([1, 128], f32)
            nc.gpsimd.memset(ones[:], 1.0)
            bcs = cp.tile([1, C0], f32)
            nc.sync.dma_start(out=bcs[:], in_=bc[:])
            pbc = pp.tile([128, C0], f32, name="pbc")
            nc.tensor.matmul(out=pbc[:], lhsT=ones[:], rhs=bcs[:],
                             start=True, stop=True)
            bcb = cp.tile([128, C0], f32, name="bcb")
            nc.vector.tensor_copy(out=bcb[:], in_=pbc[:])

            didx_r = didx[:].rearrange("(s t p) k -> s p t k", t=SUP, p=128)
            wdev_r = wdev[:].rearrange("(s t p) k -> s p t k", t=SUP, p=128)
            out_r = out[:].rearrange("(s t p) c -> s p t c", t=SUP, p=128)
            GW = KD * C0
            for s in range(NSUP_P):
                idx = sb.tile([128, SUP * KD], i32, name="idx", tag="idx")
                nc.sync.dma_start(
                    out=idx[:].rearrange("p (t k) -> p t k", t=SUP),
                    in_=didx_r[s])
                G = sb.tile([128, SUP * GW], f32, name="G", tag="G")
                for t in range(SUP):
                    for k in range(KD):
                        _gather(nc, G[:, t * GW + k * C0: t * GW + (k + 1) * C0],
                                ytab[:], idx[:, t * KD + k: t * KD + k + 1])
                w4 = sb.tile([128, SUP * KD], f32, name="w4", tag="w4")
                nc.sync.dma_start(
                    out=w4[:].rearrange("p (t k) -> p t k", t=SUP),
                    in_=wdev_r[s])
                prod = sb.tile([128, SUP * GW], f32, name="prod", tag="prod")
                gv = G[:].rearrange("p (t k c) -> p t k c", t=SUP, k=KD, c=C0)
                pvw = prod[:].rearrange("p (t c k) -> p t k c", t=SUP, c=C0, k=KD)
                wv = w4[:].rearrange("p (t k) -> p t k", t=SUP)
                wb = bass.AP(wv.tensor, wv.offset,
                             [list(wv.ap[0]), list(wv.ap[1]), list(wv.ap[2]),
                              [0, C0]])
                nc.vector.tensor_tensor(out=pvw, in0=gv, in1=wb, op=ALU.mult)
                pts = sb.tile([128, SUP * C0], f32, name="pts", tag="pts")
                nc.vector.tensor_reduce(
                    out=pts[:].rearrange("p (t c) -> p t c", t=SUP),
                    in_=prod[:].rearrange("p (t c k) -> p t c k", t=SUP, c=C0, k=KD),
                    axis=mybir.AxisListType.X, op=ALU.add)
                res = sb.tile([128, SUP * C0], f32, name="res", tag="res")
                bcv = bass.AP(bcb[:].tensor, bcb[:].offset,
                              [list(bcb[:].ap[0]), [0, SUP], list(bcb[:].ap[1])])
                nc.vector.tensor_tensor(
                    out=res[:].rearrange("p (t c) -> p t c", t=SUP),
                    in0=pts[:].rearrange("p (t c) -> p t c", t=SUP),
                    in1=bcv, op=ALU.add)
                nc.sync.dma_start(
                    out=out_r[s],
                    in_=res[:].rearrange("p (t c) -> p t c", t=SUP)[:, :, :NCLS])
    return _install_waitfix(nc)


# ---------------------------------------------------------------- host side
def _remap(g):
    g = np.asarray(g)
    gc = np.clip(g, 0, M - 1)
    s = gc // Ms
    out = s * MsP + (gc - s * Ms)
    return np.where(g < 0, ZR, out).astype(np.int32)


def _stack_w(Wk, cols):
    """W [27, cin, 32] -> padded [nchunk*128, 32] stack over (k, cin)."""
    Wk = np.asarray(Wk, np.float32)
    kcin = Wk.shape[0] * Wk.shape[1]
    nchunk = (27 * Wk.shape[1] + 127) // 128
    o = np.zeros((nchunk * 128, C0), np.float32)
    o[:kcin] = Wk.reshape(kcin, C0)
    return o


def _get_runners(dmax):
    key = ("runners", dmax)
    if key not in _cache:
        _cache[key] = {
            "vox": _Runner(build_vox(dmax)),
            "conv1": _Runner(build_conv(CIN, False)),
            "conv": _Runner(build_conv(C0, False)),
            "convr2": _Runner(build_conv(C0, True)),
            "devox": _Runner(build_devox()),
        }
    return _cache[key]


def kernel(point_fea, idx_query, nbrs, idx_dev, w_dev,
           W_s1, W_s2, g_s1, b_s1, g_s2, b_s2,
           W_r1, W_r2, g_r1, b_r1, g_r2, b_r2, W_c, b_c):
    point_fea = np.asarray(point_fea, np.float32)
    idx_query = np.asarray(idx_query, np.int32)
    nbrs = np.asarray(nbrs, np.int32)
    idx_dev = np.asarray(idx_dev, np.int32)
    w_dev = np.asarray(w_dev, np.float32)

    # ---- host preprocessing (index plumbing only)
    pf_table = np.zeros((N + 1, CIN), np.float32)
    pf_table[:N] = point_fea
    counts = np.bincount(idx_query, minlength=M)
    dmax = int(counts.max())
    order = np.argsort(idx_query, kind="stable")
    starts = np.zeros(M + 1, np.int64)
    np.cumsum(counts, out=starts[1:])
    vox_map_full = np.full((M, dmax), N, np.int32)
    pos = np.arange(N) - starts[idx_query[order]]
    vox_map_full[idx_query[order], pos] = order
    recip_full = (1.0 / np.maximum(counts, 1)).astype(np.float32)

    smat = np.zeros((dmax * CIN, CIN), np.float32)
    for d in range(dmax):
        smat[d * CIN:(d + 1) * CIN] = np.eye(CIN, dtype=np.float32)

    nb_remap = _remap(nbrs)                     # [M, 27]
    per = []
    for c in range(NC):
        vs = slice(c * Ms, (c + 1) * Ms)
        ps = slice(c * Np, (c + 1) * Np)
        vmap = np.full((MsP, dmax), N, np.int32)
        vmap[:Ms] = vox_map_full[vs]
        rcp = np.zeros((MsP, 1), np.float32)
        rcp[:Ms, 0] = recip_full[vs]
        nb28 = np.full((MsP, 27), ZR, np.int32)
        nb28[:Ms] = nb_remap[vs]
        didx = np.full((NpP, KD), ZR, np.int32)
        didx[:Np] = _remap(idx_dev[ps])
        wd = np.zeros((NpP, KD), np.float32)
        wd[:Np] = w_dev[ps]
        per.append(dict(vmap=vmap, rcp=rcp, nb28=nb28, didx=didx, wd=wd))

    W1s = _stack_w(np.asarray(W_s1), CIN)
    W2s = _stack_w(np.asarray(W_s2), C0)
    Wr1s = _stack_w(np.asarray(W_r1), C0)
    Wr2s = _stack_w(np.asarray(W_r2), C0)
    Wc_pad = np.zeros((C0, C0), np.float32)
    Wc_pad[:, :NCLS] = np.asarray(W_c)
    bc_pad = np.zeros((1, C0), np.float32)
    bc_pad[0, :NCLS] = np.asarray(b_c)

    R = _get_runners(dmax)

    def assemble(shards, zero_pads=True):
        """concat per-core [MsP, C] shards into the padded full table."""
        full = np.concatenate(shards, 0)
        if zero_pads:
            full = full.reshape(NC, MsP, -1)
            full[:, Ms:] = 0.0
            full = full.reshape(NC * MsP, -1)
        return np.ascontiguousarray(full)

    # ---- launch 1: voxelize
    res = R["vox"]([dict(pf=pf_table, vmap=per[c]["vmap"], rcp=per[c]["rcp"],
                         smat=smat) for c in range(NC)])
    vox_full = assemble([res[c]["vout"] for c in range(NC)])

    # ---- launch 2: conv1 (stem 1)
    res = R["conv1"]([dict(table=vox_full, nbrs=per[c]["nb28"], wst=W1s,
                           gpar=np.asarray(g_s1, np.float32),
                           bpar=np.asarray(b_s1, np.float32))
                      for c in range(NC)])
    h1_full = assemble([res[c]["hout"] for c in range(NC)])

    # ---- launch 3: conv2 (stem 2)
    res = R["conv"]([dict(table=h1_full, nbrs=per[c]["nb28"], wst=W2s,
                          gpar=np.asarray(g_s2, np.float32),
                          bpar=np.asarray(b_s2, np.float32))
                     for c in range(NC)])
    h2_shards = [res[c]["hout"] for c in range(NC)]
    h2_full = assemble(h2_shards)

    # ---- launch 4: r1
    res = R["conv"]([dict(table=h2_full, nbrs=per[c]["nb28"], wst=Wr1s,
                          gpar=np.asarray(g_r1, np.float32),
                          bpar=np.asarray(b_r1, np.float32))
                     for c in range(NC)])
    r1_full = assemble([res[c]["hout"] for c in range(NC)])

    # ---- launch 5: r2 + residual + classifier table
    res = R["convr2"]([dict(table=r1_full, nbrs=per[c]["nb28"], wst=Wr2s,
                            gpar=np.asarray(g_r2, np.float32),
                            bpar=np.asarray(b_r2, np.float32),
                            h2in=h2_shards[c], wc=Wc_pad)
                       for c in range(NC)])
    y_full = assemble([res[c]["hout"] for c in range(NC)])

    # ---- launch 6: devoxelize
    res = R["devox"]([dict(ytab=y_full, didx=per[c]["didx"], wdev=per[c]["wd"],
                           bc=bc_pad) for c in range(NC)])
    out = np.concatenate([res[c]["out"][:Np] for c in range(NC)], 0)
    return np.ascontiguousarray(out)

